# revision 1
# baseline (speedup 1.0000x reference)
"""Distributed Bass kernel for llama-style GQA attention on 8 trn2 NeuronCores.

Sharding: 2-way data-parallel over batch x 4-way tensor-parallel over heads.
Core c handles batch b=c//4 and head group t=c%4 (8 q-heads, 2 kv-heads).
wq/wk/wv split column-wise per head group; wo split row-wise; each core
produces a partial [S, HIDDEN] output, host sums the 4 partials per batch.

On-chip flow per core (all matmuls bf16, psum f32):
  xT (pre-transposed on host) @ wqkv -> q,k,v  [seq partition-major]
  RoPE on q (pre-scaled by 1/sqrt(D)) and k via even/odd strided APs
  PE-transpose q,k to [d, seq]; v kept [seq, d] with an appended ones column
  scores^T[k,q] = kT.T @ qT ; exp (no max subtraction -- scores are O(5));
  causal via aligned 128x128 tri mask / memset / narrowed score matmuls;
  ctx^T[d,q] accumulated with the ones column giving softmax denominators
  for free; normalize via bf16 K=1 broadcast matmul + fast reciprocal;
  out_partial = ctx^T.T @ wo_shard, split in two halves so the first half
  interleaves with attention (keeps PE dense) via a DRAM scratch.
"""

import numpy as np
import ml_dtypes

import concourse.bass as bass
import concourse.mybir as mybir
import concourse.tile as tile
from concourse import bacc
from concourse.bass_utils import run_bass_kernel_spmd
from concourse.masks import make_identity

B, S, HID = 2, 2048, 2048
D = 64
NQ, NKV = 8, 2          # per-core heads
QW, KW, VW = NQ * D, NKV * D, NKV * D
QKVW = QW + KW + VW     # 768
P = 128
SB = S // P             # 16 seq blocks
KC = HID // P           # 16 contraction chunks
F32 = mybir.dt.float32
BF16 = mybir.dt.bfloat16
BF = ml_dtypes.bfloat16
AF = mybir.ActivationFunctionType

_CACHE = {}


def _emit_graph(nc, tc, xT, wqkv, wo, cos8, sin8, cos1, sin1, out):
    with tc.tile_pool(name="const", bufs=1) as const, \
         tc.tile_pool(name="big", bufs=1) as big, \
         tc.tile_pool(name="dscr", bufs=1, space="DRAM") as dscr:
        # persistent across phases
        # q head pairs: tensor t holds head 2t dims on partitions 0:64, head 2t+1 on 64:128
        qT_sb = [big.tile([P, S], BF16, tag=f"qT{t}", name=f"qT{t}") for t in range(4)]
        # kv head k duplicated on both partition halves (so base partition matches either q half)
        kT_sb = [big.tile([P, S], BF16, tag=f"kT{k}", name=f"kT{k}") for k in range(NKV)]
        vaug_sb = big.tile([P, NKV * SB * 65], BF16, tag="va")
        ctxT_sb = [big.tile([P, S], BF16, tag=f"cT{t}", name=f"cT{t}") for t in range(4)]
        acc_dram = dscr.tile([S, HID], F32, tag="acc")

        ident = const.tile([P, P], BF16, tag="id")
        make_identity(nc, ident[:, :])
        # tri01[k, q] = 1 where q >= k else 0 (keep-mask for aligned diag blocks)
        tri01 = const.tile([P, P], BF16, tag="tri")
        nc.gpsimd.memset(tri01[:, :], 1.0)
        nc.gpsimd.affine_select(
            out=tri01[:, :], in_=tri01[:, :], compare_op=mybir.AluOpType.is_ge,
            fill=0.0, base=0, pattern=[[1, P]], channel_multiplier=-1,
        )
        ones64 = const.tile([1, D], BF16, tag="ones")
        nc.gpsimd.memset(ones64[:, :], 1.0)
        nc.gpsimd.memset(vaug_sb[:, :], 1.0)

        def rope(ps, nh, cos_t, sin_t, dst, sb, tmp_pool):
            """ps: psum [P, nh*64] pre-rotation; dst: sbuf bf16 slice [P, nh*64]."""
            half = nh * 32
            t1 = tmp_pool.tile([P, half], F32, tag="t1", name="t1")
            t2 = tmp_pool.tile([P, half], F32, tag="t2", name="t2")
            ev = ps[:, 0::2].rearrange("p (h i) -> p h i", h=nh)
            od = ps[:, 1::2].rearrange("p (h i) -> p h i", h=nh)
            c = cos_t[:, sb * 32:(sb + 1) * 32].rearrange("p (o i) -> p o i", o=1).broadcast_to((P, nh, 32))
            s = sin_t[:, sb * 32:(sb + 1) * 32].rearrange("p (o i) -> p o i", o=1).broadcast_to((P, nh, 32))
            t1r = t1[:, :].rearrange("p (h i) -> p h i", h=nh)
            t2r = t2[:, :].rearrange("p (h i) -> p h i", h=nh)
            dst_e = dst[:, 0::2].rearrange("p (h i) -> p h i", h=nh)
            dst_o = dst[:, 1::2].rearrange("p (h i) -> p h i", h=nh)
            nc.vector.tensor_mul(t1r, ev, c)
            nc.vector.tensor_mul(t2r, od, s)
            nc.vector.tensor_sub(dst_e, t1r, t2r)
            nc.vector.tensor_mul(t1r, ev, s)
            nc.vector.tensor_mul(t2r, od, c)
            nc.vector.tensor_add(dst_o, t1r, t2r)

        with tc.tile_pool(name="pss", bufs=3, space="PSUM") as pss_p, \
             tc.tile_pool(name="psc", bufs=2, space="PSUM") as psc_p, \
             tc.tile_pool(name="psb", bufs=1, space="PSUM") as psb_p, \
             tc.tile_pool(name="exs", bufs=8) as exs_p, \
             tc.tile_pool(name="nrm", bufs=4) as nrm_p:

            def attn_unit(h, qb):
                t, roff, kv = h // 2, D * (h % 2), h // 4
                qT = qT_sb[t][roff:roff + D, :]
                kT = kT_sb[kv][roff:roff + D, :]
                ctx = psc_p.tile([65, 512], F32, tag="ctx", name="ctx")
                nkb = min(4 * qb + 4, 16)
                for kb in range(nkb):
                    sT = pss_p.tile([P, 512], F32, tag="sT", name="sT")
                    j0 = max(kb - 4 * qb, 0)   # sub-blocks j < j0 are fully masked
                    nc.tensor.matmul(sT[:, j0 * P:512], kT[:, kb * P:(kb + 1) * P],
                                     qT[:, qb * 512 + j0 * P:(qb + 1) * 512], start=True, stop=True)
                    ex = exs_p.tile([P, 512], BF16, tag="ex", name="ex")
                    if j0 > 0:
                        nc.gpsimd.memset(ex[:, 0:j0 * P], 0.0)
                    nc.scalar.activation(ex[:, j0 * P:512], sT[:, j0 * P:512], AF.Exp)
                    if kb * P >= qb * 512 and j0 < 4:   # aligned diagonal sub-block
                        nc.vector.tensor_mul(ex[:, j0 * P:(j0 + 1) * P],
                                             ex[:, j0 * P:(j0 + 1) * P], tri01[:, :])
                    nc.tensor.matmul(ctx[:], vaug_sb[:, kv * SB * 65 + kb * 65: kv * SB * 65 + (kb + 1) * 65],
                                     ex[:], start=(kb == 0), stop=(kb == nkb - 1))
                # ctx rows 0:64 = unnormalized ctx dims; row 64 = softmax denominators
                den = nrm_p.tile([1, 512], BF16, tag="den", name="den")
                nc.scalar.activation(den[:], ctx[64:65, :], AF.Copy)
                bc = psb_p.tile([D, 512], F32, tag="bc", name="bc")
                nc.tensor.matmul(bc[:], ones64[:, :], den[:], start=True, stop=True)
                bcs = nrm_p.tile([D, 512], F32, tag="bcs", name="bcs")
                nc.vector.reciprocal_approx_fast(out=bcs[:], in_=bc[:])
                ntmp = nrm_p.tile([D, 512], BF16, tag="ntmp", name="ntmp")
                nc.vector.tensor_mul(ntmp[:], ctx[0:D, :], bcs[:])
                nc.sync.dma_start(out=ctxT_sb[t][roff:roff + D, qb * 512:(qb + 1) * 512],
                                  in_=ntmp[:])

            # ---- projections: kv first, then q in 2 subgroups of 4 heads ----
            # wqkv columns: [k0|k1 (128) | v0|v1 (128) | q0..q3 (256) | q4..q7 (256)]
            with tc.tile_pool(name="p1", bufs=1) as p1, \
                 tc.tile_pool(name="psg", bufs=2, space="PSUM") as psg_p, \
                 tc.tile_pool(name="rtmp", bufs=2) as rtmp_p:
                pst_p = psb_p
                xT_sb = p1.tile([P, KC * S], BF16, tag="xT")
                wqkv_sb = p1.tile([P, KC * QKVW], BF16, tag="wqkv")
                cos8_sb = p1.tile([P, SB * 32], F32, tag="c8")
                sin8_sb = p1.tile([P, SB * 32], F32, tag="s8")
                cos1_sb = p1.tile([P, SB * 32], F32, tag="c1")
                sin1_sb = p1.tile([P, SB * 32], F32, tag="s1")
                qrot_sb = p1.tile([P, SB * 256], BF16, tag="qr")
                krot_sb = p1.tile([P, SB * 128], BF16, tag="kr")
                for kc in range(KC):
                    nc.sync.dma_start(out=xT_sb[:, kc * S:(kc + 1) * S], in_=xT[kc * P:(kc + 1) * P, :])
                    nc.sync.dma_start(out=wqkv_sb[:, kc * QKVW:(kc + 1) * QKVW], in_=wqkv[kc * P:(kc + 1) * P, :])
                for sb in range(SB):
                    for dst, srcz in ((cos8_sb, cos8), (sin8_sb, sin8), (cos1_sb, cos1), (sin1_sb, sin1)):
                        nc.sync.dma_start(out=dst[:, sb * 32:(sb + 1) * 32], in_=srcz[sb * P:(sb + 1) * P, :])

                def kv_block(sb):
                    ps = psg_p.tile([P, 256], F32, tag="psg", name="psg")
                    for kc in range(KC):
                        nc.tensor.matmul(ps[:], xT_sb[:, kc * S + sb * P: kc * S + (sb + 1) * P],
                                         wqkv_sb[:, kc * QKVW: kc * QKVW + 256],
                                         start=(kc == 0), stop=(kc == KC - 1))
                    rope(ps[:, 0:KW], NKV, cos1_sb, sin1_sb, krot_sb[:, sb * KW:(sb + 1) * KW], sb, rtmp_p)
                    for kv in range(NKV):
                        nc.vector.tensor_copy(
                            vaug_sb[:, kv * SB * 65 + sb * 65: kv * SB * 65 + sb * 65 + 64],
                            ps[:, KW + kv * D: KW + (kv + 1) * D])
                    pt = pst_p.tile([P, P], BF16, tag="bc", name="pt")
                    nc.tensor.transpose(pt[:], krot_sb[:, sb * KW:(sb + 1) * KW], ident[:, :])
                    # kv0 dims land on partitions 0:64, kv1 on 64:128; write each half
                    nc.vector.tensor_copy(kT_sb[0][0:D, sb * P:(sb + 1) * P], pt[0:D, :])
                    nc.vector.tensor_copy(kT_sb[1][D:P, sb * P:(sb + 1) * P], pt[D:P, :])
                    nc.sync.dma_start(out=kT_sb[0][D:P, sb * P:(sb + 1) * P],
                                      in_=kT_sb[0][0:D, sb * P:(sb + 1) * P])
                    nc.sync.dma_start(out=kT_sb[1][0:D, sb * P:(sb + 1) * P],
                                      in_=kT_sb[1][D:P, sb * P:(sb + 1) * P])

                def q_block(j, sb):
                    ps = psg_p.tile([P, 256], F32, tag="psg", name="psg")
                    for kc in range(KC):
                        nc.tensor.matmul(ps[:], xT_sb[:, kc * S + sb * P: kc * S + (sb + 1) * P],
                                         wqkv_sb[:, kc * QKVW + 256 + j * 256: kc * QKVW + 256 + (j + 1) * 256],
                                         start=(kc == 0), stop=(kc == KC - 1))
                    rope(ps[:], 4, cos8_sb, sin8_sb, qrot_sb[:, sb * 256:(sb + 1) * 256], sb, rtmp_p)
                    for pidx in range(2):
                        t = 2 * j + pidx
                        pt = pst_p.tile([P, P], BF16, tag="bc", name="pt")
                        nc.tensor.transpose(pt[:], qrot_sb[:, sb * 256 + pidx * P: sb * 256 + (pidx + 1) * P],
                                            ident[:, :])
                        nc.vector.tensor_copy(qT_sb[t][:, sb * P:(sb + 1) * P], pt[:])

                for sb in range(SB):
                    kv_block(sb)
                for sb in range(SB):
                    q_block(0, sb)
                # q subgroup 1 interleaved with attention over subgroup-0 heads
                g0_units = [(h, qb) for h in range(4) for qb in range(4)]
                for sb in range(SB):
                    q_block(1, sb)
                    attn_unit(*g0_units[sb])

            # ---- attention subgroup 1 interleaved with first half of wo ----
            with tc.tile_pool(name="pso", bufs=2, space="PSUM") as pso_p, \
                 tc.tile_pool(name="osb", bufs=3) as osb_p, \
                 tc.tile_pool(name="wop", bufs=1) as wop:
                wo_sb = wop.tile([P, 4 * HID], BF16, tag="wo")
                for c in range(4):
                    nc.sync.dma_start(out=wo_sb[:, c * HID:(c + 1) * HID], in_=wo[c * P:(c + 1) * P, :])

                def wo_half(sb, n, cs, dst_ap, accum_sb=None):
                    po = pso_p.tile([P, 512], F32, tag="po", name="po")
                    for i, c in enumerate(cs):
                        nc.tensor.matmul(po[:], ctxT_sb[c][:, sb * P:(sb + 1) * P],
                                         wo_sb[:, c * HID + n * 512: c * HID + (n + 1) * 512],
                                         start=(i == 0), stop=(i == len(cs) - 1))
                    ob = osb_p.tile([P, 512], F32, tag="ob", name="ob")
                    if accum_sb is None:
                        nc.vector.tensor_copy(ob[:], po[:])
                    else:
                        nc.vector.tensor_add(ob[:], po[:], accum_sb[:])
                    nc.sync.dma_start(out=dst_ap, in_=ob[:])

                def wo_B(sb, n):
                    acc_sb = osb_p.tile([P, 512], F32, tag="acl", name="acl")
                    nc.sync.dma_start(out=acc_sb[:], in_=acc_dram[sb * P:(sb + 1) * P, n * 512:(n + 1) * 512])
                    wo_half(sb, n, (2, 3), out[sb * P:(sb + 1) * P, n * 512:(n + 1) * 512], accum_sb=acc_sb)

                # qb-major: after each qb's 4 heads, ctxT columns for 4 seq
                # blocks are final -> their wo halves interleave right here
                for qb in range(4):
                    for h in range(4, 8):
                        attn_unit(h, qb)
                    for sb in range(4 * qb, 4 * qb + 4):
                        for n in range(4):
                            wo_half(sb, n, (0, 1), acc_dram[sb * P:(sb + 1) * P, n * 512:(n + 1) * 512])
                    if qb > 0:
                        for sb in range(4 * (qb - 1), 4 * qb):
                            for n in range(4):
                                wo_B(sb, n)
                for sb in range(12, 16):
                    for n in range(4):
                        wo_B(sb, n)


def _build():
    nc = bacc.Bacc("TRN2", target_bir_lowering=False, debug=False, num_devices=8)
    xT = nc.dram_tensor("xT", [HID, S], BF16, kind="ExternalInput").ap()
    wqkv = nc.dram_tensor("wqkv", [HID, QKVW], BF16, kind="ExternalInput").ap()
    wo = nc.dram_tensor("wo", [QW, HID], BF16, kind="ExternalInput").ap()
    cos8 = nc.dram_tensor("cos8", [S, 32], F32, kind="ExternalInput").ap()
    sin8 = nc.dram_tensor("sin8", [S, 32], F32, kind="ExternalInput").ap()
    cos1 = nc.dram_tensor("cos1", [S, 32], F32, kind="ExternalInput").ap()
    sin1 = nc.dram_tensor("sin1", [S, 32], F32, kind="ExternalInput").ap()
    out = nc.dram_tensor("out", [S, HID], F32, kind="ExternalOutput").ap()
    with tile.TileContext(nc) as tc:
        _emit_graph(nc, tc, xT, wqkv, wo, cos8, sin8, cos1, sin1, out)
    nc.finalize()
    return nc


def kernel(x, wq, wk, wv, wo, freqs_cos, freqs_sin, mask):
    x = np.asarray(x, dtype=np.float32)
    wq = np.asarray(wq, dtype=np.float32)
    wk = np.asarray(wk, dtype=np.float32)
    wv = np.asarray(wv, dtype=np.float32)
    wo = np.asarray(wo, dtype=np.float32)
    fc = np.asarray(freqs_cos, dtype=np.float32)
    fs = np.asarray(freqs_sin, dtype=np.float32)

    if "nc" not in _CACHE:
        _CACHE["nc"] = _build()
    nc = _CACHE["nc"]

    inv = 1.0 / np.sqrt(np.float32(D))
    cos8 = np.ascontiguousarray(fc * inv)
    sin8 = np.ascontiguousarray(fs * inv)
    in_maps = []
    for core in range(8):
        b, t = core // 4, core % 4
        in_maps.append({
            "xT": np.ascontiguousarray(x[b].T).astype(BF),
            "wqkv": np.ascontiguousarray(np.concatenate(
                [wk[:, t * KW:(t + 1) * KW],
                 wv[:, t * VW:(t + 1) * VW],
                 wq[:, t * QW:(t + 1) * QW]], axis=1)).astype(BF),
            "wo": np.ascontiguousarray(wo[t * QW:(t + 1) * QW, :]).astype(BF),
            "cos8": cos8, "sin8": sin8,
            "cos1": np.ascontiguousarray(fc), "sin1": np.ascontiguousarray(fs),
        })
    trace = bool(_CACHE.get("trace"))
    try:
        res = run_bass_kernel_spmd(nc, in_maps, list(range(8)), trace=trace)
    except Exception:
        if not trace:
            raise
        res = run_bass_kernel_spmd(nc, in_maps, list(range(8)))
    _CACHE["last_result"] = res
    outs = [np.asarray(r["out"], dtype=np.float32) for r in res.results]
    full = np.stack([outs[0] + outs[1] + outs[2] + outs[3],
                     outs[4] + outs[5] + outs[6] + outs[7]], axis=0)
    return full



# revision 10
# speedup vs baseline: 1.0055x; 1.0055x over previous
"""Distributed Bass kernel for llama-style GQA attention on 8 trn2 NeuronCores.

Sharding: 2-way data-parallel over batch x 4-way tensor-parallel over heads.
Core c handles batch b=c//4 and head group t=c%4 (8 q-heads, 2 kv-heads).
wq/wk/wv split column-wise per head group; wo split row-wise; each core
produces a partial [S, HIDDEN] output (bf16), host sums the 4 partials.

Key structure (all matmuls bf16, psum f32):
  xT (pre-transposed on host) @ wqkv -> q,k,v  [seq partition-major]
  RoPE in de-interleaved layout (host permutes wq/wk columns so even/odd
  rope halves are contiguous; cos/sin replicated per head on host, bf16)
  PE-transpose q,k to [d, seq]; q head pairs share one [128, S] tile
  (head A dims on partitions 0:64, head B on 64:128); kv heads duplicated
  on both halves so score matmuls ROW-PACK: two concurrent K=64 matmuls
  at array rows 0/64 (auto tile_position from base partitions).
  exp over a [128, 1024] pair tile in ONE activation per (pair, kb);
  causal = narrowed matmul/exp/ctx ranges + gpsimd affine_select on the
  diagonal 128x128 blocks (no memsets, no masked-garbage exp).
  ctx via vaug [v|ones] M=65, pair shares the stationary operand; softmax
  denominators fall out in row 64; normalize = ACT shift-copy + K=1 ones
  matmul broadcast + DVE fast reciprocal + one DVE mul per pair.
  out = ctxT.T @ wo in one full-K pass (no DRAM round-trip); wo and the
  second q-projection are interleaved into attention's ACT-paced gaps via
  generator fillers to keep the PE dense and HAM-warm.
"""

import numpy as np
import ml_dtypes

import concourse.bass as bass
import concourse.mybir as mybir
import concourse.tile as tile
from concourse import bacc
from concourse.bass_utils import run_bass_kernel_spmd
from concourse.masks import make_identity

B, S, HID = 2, 2048, 2048
D = 64
NQ, NKV = 8, 2          # per-core heads
QW, KW, VW = NQ * D, NKV * D, NKV * D
QKVW = QW + KW + VW     # 768
P = 128
SB = S // P             # 16 seq blocks
KC = HID // P           # 16 contraction chunks
NSU = 4                 # seq super-units (4 sb each)
F32 = mybir.dt.float32
BF16 = mybir.dt.bfloat16
BF = ml_dtypes.bfloat16
AF = mybir.ActivationFunctionType
_CACHE = {}


def _emit_graph(nc, tc, xT, wqkv, wo, cosq, sinq, cosk, sink, out, dbg=None):
    with tc.tile_pool(name="const", bufs=1) as const, \
         tc.tile_pool(name="big", bufs=1) as big, \
         tc.tile_pool(name="wq_p", bufs=1) as wq_p, \
         tc.tile_pool(name="rt_p", bufs=2) as rt_p, \
         tc.tile_pool(name="exs", bufs=3) as exs_p, \
         tc.tile_pool(name="nrm", bufs=2) as nrm_p, \
         tc.tile_pool(name="osb", bufs=3) as osb_p:
        # persistent tiles
        qT_sb = [big.tile([P, S], BF16, tag=f"qT{t}", name=f"qT{t}") for t in range(4)]
        kT_sb = [big.tile([P, S], BF16, tag=f"kT{k}", name=f"kT{k}") for k in range(NKV)]
        vaug_sb = big.tile([P, NKV * SB * 65], BF16, tag="va")
        ctxT_sb = [big.tile([P, S], BF16, tag=f"cT{t}", name=f"cT{t}") for t in range(4)]

        ident = const.tile([P, P], BF16, tag="id")
        make_identity(nc, ident[:, :])
        nc.gpsimd.memset(vaug_sb[:, :], 1.0)
        ones64 = const.tile([1, D], BF16, tag="on")
        nc.gpsimd.memset(ones64[:, :], 1.0)

        xT_sb = wq_p.tile([P, KC * S], BF16, tag="xT")
        wqkv_sb = wq_p.tile([P, KC * QKVW], BF16, tag="wqkv")
        wo_sb = wq_p.tile([P, 4 * HID], BF16, tag="wo")
        cosq_sb = wq_p.tile([P, SB * 128], BF16, tag="cq")
        sinq_sb = wq_p.tile([P, SB * 128], BF16, tag="sq")
        cosk_sb = wq_p.tile([P, SB * 64], BF16, tag="ck")
        sink_sb = wq_p.tile([P, SB * 64], BF16, tag="sk")
        for kc in range(KC):
            nc.sync.dma_start(out=xT_sb[:, kc * S:(kc + 1) * S], in_=xT[kc * P:(kc + 1) * P, :])
            nc.sync.dma_start(out=wqkv_sb[:, kc * QKVW:(kc + 1) * QKVW], in_=wqkv[kc * P:(kc + 1) * P, :])
        for sb in range(SB):
            nc.sync.dma_start(out=cosq_sb[:, sb * 128:(sb + 1) * 128], in_=cosq[sb * P:(sb + 1) * P, :])
            nc.sync.dma_start(out=sinq_sb[:, sb * 128:(sb + 1) * 128], in_=sinq[sb * P:(sb + 1) * P, :])
            nc.sync.dma_start(out=cosk_sb[:, sb * 64:(sb + 1) * 64], in_=cosk[sb * P:(sb + 1) * P, :])
            nc.sync.dma_start(out=sink_sb[:, sb * 64:(sb + 1) * 64], in_=sink[sb * P:(sb + 1) * P, :])
        for c in range(4):
            nc.sync.dma_start(out=wo_sb[:, c * HID:(c + 1) * HID], in_=wo[c * P:(c + 1) * P, :])

        # ------------- rope (de-interleaved layout, 4-sb batched) -------------
        def rope4(ps, nh, cos_full, sin_full, su, dst):
            """ps: psum [P, 4*nh*64] (4 sb, nh heads, [e(32)|o(32)] per head);
            dst: sbuf bf16 same layout; cos_full: [P, SB*nh*32] bf16."""
            half = nh * 32
            n = 4 * nh            # folded (sb, head) count
            t1 = rt_p.tile([P, 4 * half], F32, tag="t1", name="t1")
            t2 = rt_p.tile([P, 4 * half], F32, tag="t2", name="t2")
            ev = ps.rearrange("p (sh x) -> p sh x", sh=n)[:, :, 0:32]
            od = ps.rearrange("p (sh x) -> p sh x", sh=n)[:, :, 32:64]
            de = dst.rearrange("p (sh x) -> p sh x", sh=n)[:, :, 0:32]
            do = dst.rearrange("p (sh x) -> p sh x", sh=n)[:, :, 32:64]
            c = cos_full[:, su * 4 * half:(su + 1) * 4 * half].rearrange(
                "p (sh i) -> p sh i", sh=n)
            sn = sin_full[:, su * 4 * half:(su + 1) * 4 * half].rearrange(
                "p (sh i) -> p sh i", sh=n)
            t1r = t1[:, :].rearrange("p (sh i) -> p sh i", sh=n)
            t2r = t2[:, :].rearrange("p (sh i) -> p sh i", sh=n)
            nc.vector.tensor_mul(t1r, ev, c)
            nc.vector.tensor_mul(t2r, od, sn)
            nc.vector.tensor_sub(de, t1r, t2r)
            nc.vector.tensor_mul(t1r, ev, sn)
            nc.vector.tensor_mul(t2r, od, c)
            nc.vector.tensor_add(do, t1r, t2r)

        # ------------- projection super-units (generators) --------------------
        def kv_su(pool, ptpool, su):
            """project k+v for 4 sb; then rope k; evac v; transpose+dup k."""
            ps = pool.tile([P, 1024], F32, tag="kv", name="kvps")
            for i in range(4):
                sb = su * 4 + i
                for kc in range(KC):
                    xc = xT_sb[:, kc * S + sb * P: kc * S + (sb + 1) * P]
                    nc.tensor.matmul(ps[:, i * 128:(i + 1) * 128], xc,
                                     wqkv_sb[:, kc * QKVW: kc * QKVW + 128],
                                     start=(kc == 0), stop=(kc == KC - 1))
                    nc.tensor.matmul(ps[:, 512 + i * 128:512 + (i + 1) * 128], xc,
                                     wqkv_sb[:, kc * QKVW + 128: kc * QKVW + 256],
                                     start=(kc == 0), stop=(kc == KC - 1))
            yield  # --- mms emitted ---
            krot = rt_p.tile([P, 512], BF16, tag="krot", name="krot")
            rope4(ps[:, 0:512], NKV, cosk_sb, sink_sb, su, krot[:, :])
            # v -> vaug (psum read, 3D view over 4 sb per kv head)
            vsrc = ps[:, 512:1024].rearrange("p (s kvd) -> p s kvd", s=4)
            for kv in range(NKV):
                vdst = vaug_sb[:, kv * SB * 65 + su * 4 * 65:
                               kv * SB * 65 + (su * 4 + 4) * 65].rearrange(
                    "p (s d) -> p s d", s=4)
                nc.vector.tensor_copy(vdst[:, :, 0:D], vsrc[:, :, kv * D:(kv + 1) * D])
            for i in range(4):
                sb = su * 4 + i
                pt = ptpool.tile([P, P], BF16, tag="pt", name="pt")
                nc.tensor.transpose(pt[:], krot[:, i * 128:(i + 1) * 128], ident[:, :])
                nc.vector.tensor_copy(kT_sb[0][0:D, sb * P:(sb + 1) * P], pt[0:D, :])
                nc.vector.tensor_copy(kT_sb[1][D:P, sb * P:(sb + 1) * P], pt[D:P, :])
                nc.sync.dma_start(out=kT_sb[0][D:P, sb * P:(sb + 1) * P],
                                  in_=kT_sb[0][0:D, sb * P:(sb + 1) * P])
                nc.sync.dma_start(out=kT_sb[1][0:D, sb * P:(sb + 1) * P],
                                  in_=kT_sb[1][D:P, sb * P:(sb + 1) * P])

        def q_su(pool, ptag, pttag, j, su, chunk):
            """project q heads 4j..4j+3 for 4 sb; rope; transpose.
            chunk = #matmuls per yield in the mm phase (filler granularity)."""
            ps = pool.tile([P, 1024], F32, tag=ptag, name="qps")
            nmm = 0
            for i in range(4):
                sb = su * 4 + i
                for kc in range(KC):
                    nc.tensor.matmul(ps[:, i * 256:(i + 1) * 256],
                                     xT_sb[:, kc * S + sb * P: kc * S + (sb + 1) * P],
                                     wqkv_sb[:, kc * QKVW + 256 + j * 256:
                                              kc * QKVW + 256 + (j + 1) * 256],
                                     start=(kc == 0), stop=(kc == KC - 1))
                    nmm += 1
                    if nmm % chunk == 0:
                        yield
            yield  # --- mms emitted ---
            qrot = rt_p.tile([P, 1024], BF16, tag="qrot", name="qrot")
            rope4(ps[:, :], 4, cosq_sb, sinq_sb, su, qrot[:, :])
            yield
            for i in range(4):
                sb = su * 4 + i
                for c in range(2):
                    t = 2 * j + c
                    pt = pool.tile([P, P], BF16, tag=pttag, name="pt")
                    nc.tensor.transpose(pt[:], qrot[:, i * 256 + c * P: i * 256 + (c + 1) * P],
                                        ident[:, :])
                    nc.vector.tensor_copy(qT_sb[t][:, sb * P:(sb + 1) * P], pt[:])
                yield

        def wo_unit(pool, sb, n):
            po = pool.tile([P, 512], F32, tag="f", name="po")
            for c in range(4):
                nc.tensor.matmul(po[:, 0:512], ctxT_sb[c][:, sb * P:(sb + 1) * P],
                                 wo_sb[:, c * HID + n * 512: c * HID + (n + 1) * 512],
                                 start=(c == 0), stop=(c == 3))
                if c == 1:
                    yield
            ob = osb_p.tile([P, 512], BF16, tag="ob", name="ob")
            nc.vector.tensor_copy(ob[:], po[:, 0:512])
            nc.sync.dma_start(out=out[sb * P:(sb + 1) * P, n * 512:(n + 1) * 512], in_=ob[:])

        def next_filler(filler):
            while filler:
                try:
                    next(filler[0])
                    return True
                except StopIteration:
                    filler.pop(0)
            return False

        # ------------- phase A: kv + q group 0 (own psum scope) ---------------
        with tc.tile_pool(name="psA", bufs=1, space="PSUM") as psA, \
             tc.tile_pool(name="psAt", bufs=2, space="PSUM") as psAt:
            kvg = [kv_su(psA, psAt, su) for su in range(NSU)]
            qg = [q_su(psA, "q", "pt", 0, su, 10 ** 9) for su in range(NSU)]

            def drain(g):
                for _ in g:
                    pass

            next(kvg[0])
            next(qg[0])
            for su in range(NSU):
                drain(kvg[su])
                if su + 1 < NSU:
                    next(kvg[su + 1])
                drain(qg[su])
                if su + 1 < NSU:
                    next(qg[su + 1])

        # ------------- attention + qg1 + wo, interleaved ----------------------
        with tc.tile_pool(name="pss", bufs=2, space="PSUM") as pss_p, \
             tc.tile_pool(name="psc", bufs=1, space="PSUM") as psc_p, \
             tc.tile_pool(name="psF", bufs=1, space="PSUM") as psF:

            def attn_unit(t, qb, filler):
                """pair t (heads 2t, 2t+1), q block qb (512 wide)."""
                kv = t // 2
                nkb = min(4 * qb + 4, SB)
                ctx = psc_p.tile([P, 1024], F32, tag="ctx", name="ctx")
                for kb in range(nkb):
                    w0 = max(kb - 4 * qb, 0) * P
                    sps = pss_p.tile([P, 1024], F32, tag="sT", name="sT")
                    nc.tensor.matmul(sps[:, w0:512],
                                     kT_sb[kv][0:D, kb * P:(kb + 1) * P],
                                     qT_sb[t][0:D, qb * 512 + w0:(qb + 1) * 512],
                                     start=True, stop=True)
                    nc.tensor.matmul(sps[:, 512 + w0:1024],
                                     kT_sb[kv][D:P, kb * P:(kb + 1) * P],
                                     qT_sb[t][D:P, qb * 512 + w0:(qb + 1) * 512],
                                     start=True, stop=True)
                    ex = exs_p.tile([P, 1024], BF16, tag="ex", name="ex")
                    nc.scalar.activation(
                        ex.rearrange("p (h q) -> p h q", h=2)[:, :, w0:512],
                        sps.rearrange("p (h q) -> p h q", h=2)[:, :, w0:512],
                        AF.Exp)
                    if kb >= 4 * qb:     # diagonal 128x128 sub-block
                        jc = (kb - 4 * qb) * P
                        for h in range(2):
                            nc.gpsimd.affine_select(
                                out=ex[:, h * 512 + jc:h * 512 + jc + P],
                                in_=ex[:, h * 512 + jc:h * 512 + jc + P],
                                compare_op=mybir.AluOpType.is_ge,
                                fill=0.0, base=0, pattern=[[1, P]], channel_multiplier=-1)
                    next_filler(filler)
                    next_filler(filler)
                    va = vaug_sb[:, kv * SB * 65 + kb * 65: kv * SB * 65 + kb * 65 + 65]
                    nc.tensor.matmul(ctx[0:65, w0:512], va, ex[:, w0:512],
                                     start=(kb == 0), stop=(kb == nkb - 1))
                    nc.tensor.matmul(ctx[0:65, 512 + w0:1024], va, ex[:, 512 + w0:1024],
                                     start=(kb == 0), stop=(kb == nkb - 1))
                # normalize: den = ctx row 64. ACT copy shifts it to partition
                # 0 (bf16); K=1 ones-matmul broadcasts to 64 psum rows; DVE
                # recip + mul at base partition 0 (custom DVE ops and gpsimd
                # partition_broadcast misbehave at base != 0 on HW).
                den = nrm_p.tile([1, 1024], BF16, tag="den", name="den")
                nc.scalar.activation(den[0:1, :], ctx[64:65, :], AF.Copy)
                bz = psF.tile([P, 1024], F32, tag="f", name="bz")
                nc.tensor.matmul(bz[0:D, 0:512], ones64[0:1, :], den[0:1, 0:512],
                                 start=True, stop=True)
                nc.tensor.matmul(bz[0:D, 512:1024], ones64[0:1, :], den[0:1, 512:1024],
                                 start=True, stop=True)
                bcs = nrm_p.tile([D, 1024], F32, tag="bcs", name="bcs")
                nc.vector.reciprocal_approx_fast(out=bcs[0:D, :], in_=bz[0:D, :])
                ntmp = nrm_p.tile([D, 1024], BF16, tag="ntmp", name="ntmp")
                nc.vector.tensor_mul(ntmp[:, :], ctx[0:D, :], bcs[0:D, :])
                nc.sync.dma_start(out=ctxT_sb[t][0:D, qb * 512:(qb + 1) * 512],
                                  in_=ntmp[:, 0:512])
                nc.sync.dma_start(out=ctxT_sb[t][D:P, qb * 512:(qb + 1) * 512],
                                  in_=ntmp[:, 512:1024])

            filler = [q_su(psF, "f", "f", 1, su, 8) for su in range(NSU)]
            order = [(0, 0), (1, 0), (0, 1), (1, 1), (2, 0), (3, 0),
                     (0, 2), (1, 2), (2, 1), (3, 1), (0, 3), (1, 3),
                     (2, 2), (3, 2), (2, 3), (3, 3)]
            done_qb = [0, 0, 0, 0]
            for t, qb in order:
                attn_unit(t, qb, filler)
                done_qb[qb] += 1
                if done_qb[qb] == 4:    # all pairs finished this qb -> wo ready
                    for n in range(4):
                        for sb in range(4 * qb, 4 * qb + 4):
                            filler.append(wo_unit(psF, sb, n))
            while next_filler(filler):
                pass

        if dbg:
            for t in range(4):
                nc.sync.dma_start(out=dbg[f"qT{t}"], in_=qT_sb[t][:, :])
                nc.sync.dma_start(out=dbg[f"cT{t}"], in_=ctxT_sb[t][:, :])
            for k in range(NKV):
                nc.sync.dma_start(out=dbg[f"kT{k}"], in_=kT_sb[k][:, :])
            nc.sync.dma_start(out=dbg["va"], in_=vaug_sb[:, :])


def _build(debug=False):
    nc = bacc.Bacc("TRN2", target_bir_lowering=False, debug=False, num_devices=8)
    xT = nc.dram_tensor("xT", [HID, S], BF16, kind="ExternalInput").ap()
    wqkv = nc.dram_tensor("wqkv", [HID, QKVW], BF16, kind="ExternalInput").ap()
    wo = nc.dram_tensor("wo", [QW, HID], BF16, kind="ExternalInput").ap()
    cosq = nc.dram_tensor("cosq", [S, 128], BF16, kind="ExternalInput").ap()
    sinq = nc.dram_tensor("sinq", [S, 128], BF16, kind="ExternalInput").ap()
    cosk = nc.dram_tensor("cosk", [S, 64], BF16, kind="ExternalInput").ap()
    sink = nc.dram_tensor("sink", [S, 64], BF16, kind="ExternalInput").ap()
    out = nc.dram_tensor("out", [S, HID], BF16, kind="ExternalOutput").ap()
    dbg = None
    if debug:
        dbg = {}
        for t in range(4):
            dbg[f"qT{t}"] = nc.dram_tensor(f"dbg_qT{t}", [P, S], BF16, kind="ExternalOutput").ap()
            dbg[f"cT{t}"] = nc.dram_tensor(f"dbg_cT{t}", [P, S], BF16, kind="ExternalOutput").ap()
        for k in range(NKV):
            dbg[f"kT{k}"] = nc.dram_tensor(f"dbg_kT{k}", [P, S], BF16, kind="ExternalOutput").ap()
        dbg["va"] = nc.dram_tensor("dbg_va", [P, NKV * SB * 65], BF16, kind="ExternalOutput").ap()
    with tile.TileContext(nc) as tc:
        _emit_graph(nc, tc, xT, wqkv, wo, cosq, sinq, cosk, sink, out, dbg)
    nc.finalize()
    return nc


_DEINT = np.concatenate([np.arange(0, D, 2), np.arange(1, D, 2)])  # per-head perm


def _deint_cols(w, nheads):
    """permute last-dim columns: per head, evens then odds."""
    cols = np.concatenate([h * D + _DEINT for h in range(nheads)])
    return w[:, cols]


def kernel(x, wq, wk, wv, wo, freqs_cos, freqs_sin, mask):
    x = np.asarray(x, dtype=np.float32)
    wq = np.asarray(wq, dtype=np.float32)
    wk = np.asarray(wk, dtype=np.float32)
    wv = np.asarray(wv, dtype=np.float32)
    wo = np.asarray(wo, dtype=np.float32)
    fc = np.asarray(freqs_cos, dtype=np.float32)
    fs = np.asarray(freqs_sin, dtype=np.float32)

    if "nc" not in _CACHE:
        _CACHE["nc"] = _build()
    nc = _CACHE["nc"]

    inv = np.float32(1.0 / np.sqrt(np.float32(D)))
    cosq = np.ascontiguousarray(np.tile(fc * inv, (1, 4))).astype(BF)
    sinq = np.ascontiguousarray(np.tile(fs * inv, (1, 4))).astype(BF)
    cosk = np.ascontiguousarray(np.tile(fc, (1, 2))).astype(BF)
    sink = np.ascontiguousarray(np.tile(fs, (1, 2))).astype(BF)
    in_maps = []
    for core in range(8):
        b, t = core // 4, core % 4
        in_maps.append({
            "xT": np.ascontiguousarray(x[b].T).astype(BF),
            "wqkv": np.ascontiguousarray(np.concatenate(
                [_deint_cols(wk[:, t * KW:(t + 1) * KW], NKV),
                 wv[:, t * VW:(t + 1) * VW],
                 _deint_cols(wq[:, t * QW:(t + 1) * QW], NQ)], axis=1)).astype(BF),
            "wo": np.ascontiguousarray(wo[t * QW:(t + 1) * QW, :]).astype(BF),
            "cosq": cosq, "sinq": sinq, "cosk": cosk, "sink": sink,
        })
    trace = bool(_CACHE.get("trace"))
    try:
        res = run_bass_kernel_spmd(nc, in_maps, list(range(8)), trace=trace)
    except Exception:
        if not trace:
            raise
        res = run_bass_kernel_spmd(nc, in_maps, list(range(8)))
    _CACHE["last_result"] = res
    outs = [np.asarray(r["out"], dtype=np.float32) for r in res.results]
    full = np.stack([outs[0] + outs[1] + outs[2] + outs[3],
                     outs[4] + outs[5] + outs[6] + outs[7]], axis=0)
    return full


if __name__ == "__main__":
    import sys
    if "--build" in sys.argv:
        _build()
        print("build OK")


# revision 15
# speedup vs baseline: 1.0118x; 1.0063x over previous
"""Distributed Bass kernel for llama-style GQA attention on 8 trn2 NeuronCores.

Sharding: 2-way data-parallel over batch x 4-way tensor-parallel over heads.
Core c handles batch b=c//4 and head group t=c%4 (8 q-heads, 2 kv-heads).
wq/wk/wv split column-wise per head group; wo split row-wise; each core
produces a partial [S, HIDDEN] output (bf16), host sums the 4 partials.

Key structure (all matmuls bf16, psum f32):
  xT (pre-transposed on host) @ wqkv -> q,k,v  [seq partition-major].
  Phase A is DMA-paced chunk-major: as each xT/wqkv hidden-chunk arrives,
  the kv projections of the first two 4-seq-block super-units and the q
  projections of the first advance, so the PE tracks the DMA stream.
  RoPE in de-interleaved layout (host permutes wq/wk columns so even/odd
  rope halves are contiguous; cos/sin replicated per head on host, bf16).
  PE-transpose q,k to [d, seq]; q head pairs share one [128, S] tile
  (head A dims on partitions 0:64, head B on 64:128); kv heads duplicated
  on both halves so score matmuls ROW-PACK: two concurrent K=64 matmuls
  at array rows 0/64 (auto tile_position from base partitions).
  exp over a [128, 1024] pair tile in ONE activation per (pair, kb);
  causal = narrowed matmul/exp/ctx ranges + gpsimd affine_select on the
  diagonal 128x128 blocks (no memsets, no masked-garbage exp).
  ctx via vaug [v|ones] M=65, the pair shares the stationary operand;
  softmax denominators fall out in row 64; normalize = ACT shift-copy of
  the den row to partition 0 + K=1 ones-matmul broadcast + DVE fast
  reciprocal + one DVE mul (custom DVE ops and gpsimd partition ops
  misbehave at base partition != 0 on HW).
  out = ctxT.T @ wo in one full-K pass (no DRAM round-trip); wo and the
  second q-projection are interleaved one step per attention kb via
  generator fillers to keep the PE dense and HAM-warm; the wo tail
  alternates psum pools and evacuation engines to stay dense.
"""

import numpy as np
import ml_dtypes

import concourse.bass as bass
import concourse.mybir as mybir
import concourse.tile as tile
from concourse import bacc
from concourse.bass_utils import run_bass_kernel_spmd
from concourse.masks import make_identity

B, S, HID = 2, 2048, 2048
D = 64
NQ, NKV = 8, 2          # per-core heads
QW, KW, VW = NQ * D, NKV * D, NKV * D
QKVW = QW + KW + VW     # 768
P = 128
SB = S // P             # 16 seq blocks
KC = HID // P           # 16 contraction chunks
NSU = 4                 # seq super-units (4 sb each)
F32 = mybir.dt.float32
BF16 = mybir.dt.bfloat16
BF = ml_dtypes.bfloat16
AF = mybir.ActivationFunctionType
_CACHE = {}


def _emit_graph(nc, tc, xT, wqkv, wo, cosq, sinq, cosk, sink, out, dbg=None):
    with tc.tile_pool(name="const", bufs=1) as const, \
         tc.tile_pool(name="big", bufs=1) as big, \
         tc.tile_pool(name="wq_p", bufs=1) as wq_p, \
         tc.tile_pool(name="rt_p", bufs=2) as rt_p, \
         tc.tile_pool(name="exs", bufs=3) as exs_p, \
         tc.tile_pool(name="nrm", bufs=2) as nrm_p, \
         tc.tile_pool(name="osb", bufs=3) as osb_p:
        # persistent tiles
        qT_sb = [big.tile([P, S], BF16, tag=f"qT{t}", name=f"qT{t}") for t in range(4)]
        kT_sb = [big.tile([P, S], BF16, tag=f"kT{k}", name=f"kT{k}") for k in range(NKV)]
        vaug_sb = big.tile([P, NKV * SB * 65], BF16, tag="va")
        ctxT_sb = [big.tile([P, S], BF16, tag=f"cT{t}", name=f"cT{t}") for t in range(4)]

        ident = const.tile([P, P], BF16, tag="id")
        make_identity(nc, ident[:, :])
        nc.gpsimd.memset(vaug_sb[:, :], 1.0)
        ones64 = const.tile([1, D], BF16, tag="on")
        nc.gpsimd.memset(ones64[:, :], 1.0)

        xT_sb = wq_p.tile([P, KC * S], BF16, tag="xT")
        wqkv_sb = wq_p.tile([P, KC * QKVW], BF16, tag="wqkv")
        wo_sb = wq_p.tile([P, 4 * HID], BF16, tag="wo")
        cosq_sb = wq_p.tile([P, SB * 128], BF16, tag="cq")
        sinq_sb = wq_p.tile([P, SB * 128], BF16, tag="sq")
        cosk_sb = wq_p.tile([P, SB * 64], BF16, tag="ck")
        sink_sb = wq_p.tile([P, SB * 64], BF16, tag="sk")
        # critical-path DMAs first: per hidden-chunk, xT then the kv+qg0
        # slice of wqkv; everything else queued after.
        for kc in range(KC):
            nc.sync.dma_start(out=xT_sb[:, kc * S:(kc + 1) * S], in_=xT[kc * P:(kc + 1) * P, :])
            nc.sync.dma_start(out=wqkv_sb[:, kc * QKVW:kc * QKVW + 512],
                              in_=wqkv[kc * P:(kc + 1) * P, 0:512])
        for sb in range(SB):
            nc.sync.dma_start(out=cosk_sb[:, sb * 64:(sb + 1) * 64], in_=cosk[sb * P:(sb + 1) * P, :])
            nc.sync.dma_start(out=sink_sb[:, sb * 64:(sb + 1) * 64], in_=sink[sb * P:(sb + 1) * P, :])
            nc.sync.dma_start(out=cosq_sb[:, sb * 128:(sb + 1) * 128], in_=cosq[sb * P:(sb + 1) * P, :])
            nc.sync.dma_start(out=sinq_sb[:, sb * 128:(sb + 1) * 128], in_=sinq[sb * P:(sb + 1) * P, :])
        for kc in range(KC):
            nc.sync.dma_start(out=wqkv_sb[:, kc * QKVW + 512:(kc + 1) * QKVW],
                              in_=wqkv[kc * P:(kc + 1) * P, 512:QKVW])
        for c in range(4):
            nc.sync.dma_start(out=wo_sb[:, c * HID:(c + 1) * HID], in_=wo[c * P:(c + 1) * P, :])

        # ------------- rope (de-interleaved layout, 4-sb batched) -------------
        def rope_q(ps, su, dst):
            """ps: psum [P, 1024] (4 sb x 4 heads x [e|o]32); dst same, bf16."""
            n = 16
            t1 = rt_p.tile([P, 512], F32, tag="t1", name="t1")
            t2 = rt_p.tile([P, 512], F32, tag="t2", name="t2")
            ev = ps.rearrange("p (sh x) -> p sh x", sh=n)[:, :, 0:32]
            od = ps.rearrange("p (sh x) -> p sh x", sh=n)[:, :, 32:64]
            de = dst.rearrange("p (sh x) -> p sh x", sh=n)[:, :, 0:32]
            do = dst.rearrange("p (sh x) -> p sh x", sh=n)[:, :, 32:64]
            c = cosq_sb[:, su * 512:(su + 1) * 512].rearrange("p (sh i) -> p sh i", sh=n)
            sn = sinq_sb[:, su * 512:(su + 1) * 512].rearrange("p (sh i) -> p sh i", sh=n)
            t1r = t1[:, :].rearrange("p (sh i) -> p sh i", sh=n)
            t2r = t2[:, :].rearrange("p (sh i) -> p sh i", sh=n)
            nc.vector.tensor_mul(t1r, ev, c)
            nc.vector.tensor_mul(t2r, od, sn)
            nc.vector.tensor_sub(de, t1r, t2r)
            nc.vector.tensor_mul(t1r, ev, sn)
            nc.vector.tensor_mul(t2r, od, c)
            nc.vector.tensor_add(do, t1r, t2r)

        def rope_k(ps, su, dst):
            """ps: psum [P, 1024] (4 sb x [k: 2 kv x [e|o]32 | v: 128]);
            dst: krot [P, 512] (4 sb x 2 kv x 64), bf16. 4D views."""
            t1 = rt_p.tile([P, 256], F32, tag="t1k", name="t1k")
            t2 = rt_p.tile([P, 256], F32, tag="t2k", name="t2k")
            kview = ps.rearrange("p (s c) -> p s c", s=4)
            ev = kview[:, :, 0:128].rearrange("p s (h x) -> p s h x", h=2)[:, :, :, 0:32]
            od = kview[:, :, 0:128].rearrange("p s (h x) -> p s h x", h=2)[:, :, :, 32:64]
            dv = dst.rearrange("p (s c) -> p s c", s=4)
            de = dv.rearrange("p s (h x) -> p s h x", h=2)[:, :, :, 0:32]
            do = dv.rearrange("p s (h x) -> p s h x", h=2)[:, :, :, 32:64]
            c = cosk_sb[:, su * 256:(su + 1) * 256].rearrange("p (s h i) -> p s h i", s=4, h=2)
            sn = sink_sb[:, su * 256:(su + 1) * 256].rearrange("p (s h i) -> p s h i", s=4, h=2)
            t1r = t1[:, :].rearrange("p (s h i) -> p s h i", s=4, h=2)
            t2r = t2[:, :].rearrange("p (s h i) -> p s h i", s=4, h=2)
            nc.vector.tensor_mul(t1r, ev, c)
            nc.vector.tensor_mul(t2r, od, sn)
            nc.vector.tensor_sub(de, t1r, t2r)
            nc.vector.tensor_mul(t1r, ev, sn)
            nc.vector.tensor_mul(t2r, od, c)
            nc.vector.tensor_add(do, t1r, t2r)

        # ------------- projection pieces --------------------------------------
        # NOTE: at most ONE open psum accumulation group per bank. A [P, 1024]
        # su tile spans 2 banks (regions i=0,1 in bank 0; i=2,3 in bank 1), so
        # chunk-major passes interleave regions {0, 2} then {1, 3}.
        def kv_mm_one(ps, su, i, kc):
            sb = su * 4 + i
            nc.tensor.matmul(ps[:, i * 256:(i + 1) * 256],
                             xT_sb[:, kc * S + sb * P: kc * S + (sb + 1) * P],
                             wqkv_sb[:, kc * QKVW: kc * QKVW + 256],
                             start=(kc == 0), stop=(kc == KC - 1))

        def q_mm_one(ps, j, su, i, kc):
            sb = su * 4 + i
            nc.tensor.matmul(ps[:, i * 256:(i + 1) * 256],
                             xT_sb[:, kc * S + sb * P: kc * S + (sb + 1) * P],
                             wqkv_sb[:, kc * QKVW + 256 + j * 256:
                                      kc * QKVW + 256 + (j + 1) * 256],
                             start=(kc == 0), stop=(kc == KC - 1))

        def kv_mms(ps, su):
            for i in range(4):
                for kc in range(KC):
                    kv_mm_one(ps, su, i, kc)

        def q_mms(ps, j, su):
            for i in range(4):
                for kc in range(KC):
                    q_mm_one(ps, j, su, i, kc)

        def kv_side(ps, su):
            """rope k; evac v (DVE)."""
            krot = rt_p.tile([P, 512], BF16, tag="krot", name="krot")
            rope_k(ps, su, krot[:, :])
            vsrc = ps.rearrange("p (s c) -> p s c", s=4)
            for kv in range(NKV):
                vdst = vaug_sb[:, kv * SB * 65 + su * 4 * 65:
                               kv * SB * 65 + (su * 4 + 4) * 65].rearrange(
                    "p (s d) -> p s d", s=4)
                nc.vector.tensor_copy(vdst[:, :, 0:D],
                                      vsrc[:, :, 128 + kv * D:128 + (kv + 1) * D])
            return krot

        def kv_transp(krot, su, ptpool, pttag):
            for i in range(4):
                sb = su * 4 + i
                pt = ptpool.tile([P, P], BF16, tag=pttag, name="pt")
                nc.tensor.transpose(pt[:], krot[:, i * 128:(i + 1) * 128], ident[:, :])
                nc.vector.tensor_copy(kT_sb[0][0:D, sb * P:(sb + 1) * P], pt[0:D, :])
                nc.vector.tensor_copy(kT_sb[1][D:P, sb * P:(sb + 1) * P], pt[D:P, :])
                nc.sync.dma_start(out=kT_sb[0][D:P, sb * P:(sb + 1) * P],
                                  in_=kT_sb[0][0:D, sb * P:(sb + 1) * P])
                nc.sync.dma_start(out=kT_sb[1][0:D, sb * P:(sb + 1) * P],
                                  in_=kT_sb[1][D:P, sb * P:(sb + 1) * P])

        def q_side(ps, su):
            qrot = rt_p.tile([P, 1024], BF16, tag="qrot", name="qrot")
            rope_q(ps, su, qrot[:, :])
            return qrot

        def q_transp(qrot, j, su, ptpool, pttag):
            for i in range(4):
                sb = su * 4 + i
                for c in range(2):
                    t = 2 * j + c
                    pt = ptpool.tile([P, P], BF16, tag=pttag, name="pt")
                    nc.tensor.transpose(pt[:], qrot[:, i * 256 + c * P: i * 256 + (c + 1) * P],
                                        ident[:, :])
                    nc.vector.tensor_copy(qT_sb[t][:, sb * P:(sb + 1) * P], pt[:])

        def q_su_filler(pool, j, su, chunk=8):
            """generator: q projection super-unit as attention filler."""
            ps = pool.tile([P, 1024], F32, tag="f", name="qps")
            nmm = 0
            for i in range(4):
                for kc in range(KC):
                    q_mm_one(ps, j, su, i, kc)
                    nmm += 1
                    if nmm % chunk == 0:
                        yield
            qrot = q_side(ps, su)
            # let the rope (DVE) drain while attention proceeds before
            # emitting PE transposes that wait on it
            for _ in range(6):
                yield
            for i in range(4):
                sb = su * 4 + i
                for c in range(2):
                    t = 2 * j + c
                    pt = pool.tile([P, P], BF16, tag="f", name="pt")
                    nc.tensor.transpose(pt[:], qrot[:, i * 256 + c * P: i * 256 + (c + 1) * P],
                                        ident[:, :])
                    nc.vector.tensor_copy(qT_sb[t][:, sb * P:(sb + 1) * P], pt[:])
                yield

        def wo_unit(pool, tag, sb, n, evac):
            po = pool.tile([P, 1024], F32, tag=tag, name="po")
            for c in range(4):
                nc.tensor.matmul(po[:, 0:512], ctxT_sb[c][:, sb * P:(sb + 1) * P],
                                 wo_sb[:, c * HID + n * 512: c * HID + (n + 1) * 512],
                                 start=(c == 0), stop=(c == 3))
                if c == 1:
                    yield
            ob = osb_p.tile([P, 512], BF16, tag="ob", name="ob")
            if evac == "s":
                nc.scalar.activation(ob[:, :], po[:, 0:512], AF.Copy)
            else:
                nc.vector.tensor_copy(ob[:], po[:, 0:512])
            nc.sync.dma_start(out=out[sb * P:(sb + 1) * P, n * 512:(n + 1) * 512], in_=ob[:])

        def next_filler(filler):
            while filler:
                try:
                    next(filler[0])
                    return True
                except StopIteration:
                    filler.pop(0)
            return False

        # ------------- phase A ------------------------------------------------
        with tc.tile_pool(name="psA", bufs=1, space="PSUM") as psA, \
             tc.tile_pool(name="psAt", bufs=2, space="PSUM") as psAt:
            kvp0 = psA.tile([P, 1024], F32, tag="kv0", name="kvps0")
            kvp1 = psA.tile([P, 1024], F32, tag="kv1", name="kvps1")
            qp0 = psA.tile([P, 1024], F32, tag="q", name="qps0")
            # pass 1: chunk-major over regions {0, 2} (one open group per
            # bank) — the PE tracks the xT/wqkv DMA arrival order
            for kc in range(KC):
                for ps_, su in ((kvp0, 0), (kvp1, 1)):
                    kv_mm_one(ps_, su, 0, kc)
                    kv_mm_one(ps_, su, 2, kc)
                q_mm_one(qp0, 0, 0, 0, kc)
                q_mm_one(qp0, 0, 0, 2, kc)
            # pass 2: regions {1, 3}, all chunks resident -> dense
            for kc in range(KC):
                for ps_, su in ((kvp0, 0), (kvp1, 1)):
                    kv_mm_one(ps_, su, 1, kc)
                    kv_mm_one(ps_, su, 3, kc)
                q_mm_one(qp0, 0, 0, 1, kc)
                q_mm_one(qp0, 0, 0, 3, kc)
            kr0 = kv_side(kvp0, 0)            # DVE-only
            qr0 = q_side(qp0, 0)
            kvp2 = psA.tile([P, 1024], F32, tag="kv0", name="kvps2")
            kv_mms(kvp2, 2)                   # PE dense while ropes drain
            kr1 = kv_side(kvp1, 1)
            qp1 = psA.tile([P, 1024], F32, tag="q", name="qps1")
            q_mms(qp1, 0, 1)
            kv_transp(kr0, 0, psAt, "pt")
            kv_transp(kr1, 1, psAt, "pt")
            q_transp(qr0, 0, 0, psAt, "pt")
            kvp3 = psA.tile([P, 1024], F32, tag="kv1", name="kvps3")
            kv_mms(kvp3, 3)
            kr2 = kv_side(kvp2, 2)
            qr1 = q_side(qp1, 1)
            qp2 = psA.tile([P, 1024], F32, tag="q", name="qps2")
            q_mms(qp2, 0, 2)
            kv_transp(kr2, 2, psAt, "pt")
            q_transp(qr1, 0, 1, psAt, "pt")
            kr3 = kv_side(kvp3, 3)
            qr2 = q_side(qp2, 2)
            qp3 = psA.tile([P, 1024], F32, tag="q", name="qps3")
            q_mms(qp3, 0, 3)
            kv_transp(kr3, 3, psAt, "pt")
            q_transp(qr2, 0, 2, psAt, "pt")
            qr3 = q_side(qp3, 3)
            q_transp(qr3, 0, 3, psAt, "pt")

        # ------------- attention + qg1 + wo, interleaved ----------------------
        with tc.tile_pool(name="pss", bufs=2, space="PSUM") as pss_p, \
             tc.tile_pool(name="psc", bufs=1, space="PSUM") as psc_p, \
             tc.tile_pool(name="psF", bufs=1, space="PSUM") as psF:

            def attn_unit(t, qb, filler):
                """pair t (heads 2t, 2t+1), q block qb (512 wide)."""
                kv = t // 2
                nkb = min(4 * qb + 4, SB)
                ctx = psc_p.tile([P, 1024], F32, tag="ctx", name="ctx")
                for kb in range(nkb):
                    w0 = max(kb - 4 * qb, 0) * P
                    sps = pss_p.tile([P, 1024], F32, tag="sT", name="sT")
                    nc.tensor.matmul(sps[:, w0:512],
                                     kT_sb[kv][0:D, kb * P:(kb + 1) * P],
                                     qT_sb[t][0:D, qb * 512 + w0:(qb + 1) * 512],
                                     start=True, stop=True)
                    nc.tensor.matmul(sps[:, 512 + w0:1024],
                                     kT_sb[kv][D:P, kb * P:(kb + 1) * P],
                                     qT_sb[t][D:P, qb * 512 + w0:(qb + 1) * 512],
                                     start=True, stop=True)
                    ex = exs_p.tile([P, 1024], BF16, tag="ex", name="ex")
                    nc.scalar.activation(
                        ex.rearrange("p (h q) -> p h q", h=2)[:, :, w0:512],
                        sps.rearrange("p (h q) -> p h q", h=2)[:, :, w0:512],
                        AF.Exp)
                    if kb >= 4 * qb:     # diagonal 128x128 sub-block
                        jc = (kb - 4 * qb) * P
                        for h in range(2):
                            nc.gpsimd.affine_select(
                                out=ex[:, h * 512 + jc:h * 512 + jc + P],
                                in_=ex[:, h * 512 + jc:h * 512 + jc + P],
                                compare_op=mybir.AluOpType.is_ge,
                                fill=0.0, base=0, pattern=[[1, P]], channel_multiplier=-1)
                    next_filler(filler)
                    va = vaug_sb[:, kv * SB * 65 + kb * 65: kv * SB * 65 + kb * 65 + 65]
                    nc.tensor.matmul(ctx[0:65, w0:512], va, ex[:, w0:512],
                                     start=(kb == 0), stop=(kb == nkb - 1))
                    nc.tensor.matmul(ctx[0:65, 512 + w0:1024], va, ex[:, 512 + w0:1024],
                                     start=(kb == 0), stop=(kb == nkb - 1))
                # normalize: den = ctx row 64 -> ACT shift to partition 0 ->
                # ones-matmul broadcast -> DVE recip + mul (base 0 only).
                den = nrm_p.tile([1, 1024], BF16, tag="den", name="den")
                nc.scalar.activation(den[0:1, :], ctx[64:65, :], AF.Copy)
                # bz lives in the scores ring: its prior occupant's readers
                # (exp) are always already emitted, unlike the psF ring whose
                # occupant may be a filler projection awaiting its rope.
                bz = pss_p.tile([P, 1024], F32, tag="sT", name="bz")
                nc.tensor.matmul(bz[0:D, 0:512], ones64[0:1, :], den[0:1, 0:512],
                                 start=True, stop=True)
                nc.tensor.matmul(bz[0:D, 512:1024], ones64[0:1, :], den[0:1, 512:1024],
                                 start=True, stop=True)
                bcs = nrm_p.tile([D, 1024], F32, tag="bcs", name="bcs")
                nc.vector.reciprocal_approx_fast(out=bcs[0:D, :], in_=bz[0:D, :])
                ntmp = nrm_p.tile([D, 1024], BF16, tag="ntmp", name="ntmp")
                nc.vector.tensor_mul(ntmp[:, :], ctx[0:D, :], bcs[0:D, :])
                nc.sync.dma_start(out=ctxT_sb[t][0:D, qb * 512:(qb + 1) * 512],
                                  in_=ntmp[:, 0:512])
                nc.sync.dma_start(out=ctxT_sb[t][D:P, qb * 512:(qb + 1) * 512],
                                  in_=ntmp[:, 512:1024])

            filler = [q_su_filler(psF, 1, su) for su in range(NSU)]
            order = [(0, 0), (1, 0), (0, 1), (1, 1), (2, 0), (3, 0),
                     (0, 2), (1, 2), (2, 1), (3, 1), (2, 2), (3, 2),
                     (0, 3), (1, 3), (2, 3), (3, 3)]
            done_qb = [0, 0, 0, 0]
            for t, qb in order:
                attn_unit(t, qb, filler)
                done_qb[qb] += 1
                if done_qb[qb] == 4 and qb < 3:
                    for n in range(4):
                        for sb in range(4 * qb, 4 * qb + 4):
                            filler.append(wo_unit(psF, "f", sb, n, "v"))
            while next_filler(filler):
                pass
            # dense tail: last qb's wo alternates psum pool + evac engine
            tail = []
            for i, (n, sb) in enumerate([(n, sb) for n in range(4) for sb in range(12, 16)]):
                pool, tag = (psF, "f") if i % 2 == 0 else (pss_p, "sT")
                tail.append(wo_unit(pool, tag, sb, n, "v" if i % 2 == 0 else "s"))
            while next_filler(tail):
                pass

        if dbg:
            for t in range(4):
                nc.sync.dma_start(out=dbg[f"qT{t}"], in_=qT_sb[t][:, :])
                nc.sync.dma_start(out=dbg[f"cT{t}"], in_=ctxT_sb[t][:, :])
            for k in range(NKV):
                nc.sync.dma_start(out=dbg[f"kT{k}"], in_=kT_sb[k][:, :])
            nc.sync.dma_start(out=dbg["va"], in_=vaug_sb[:, :])


def _build(debug=False):
    nc = bacc.Bacc("TRN2", target_bir_lowering=False, debug=False, num_devices=8)
    xT = nc.dram_tensor("xT", [HID, S], BF16, kind="ExternalInput").ap()
    wqkv = nc.dram_tensor("wqkv", [HID, QKVW], BF16, kind="ExternalInput").ap()
    wo = nc.dram_tensor("wo", [QW, HID], BF16, kind="ExternalInput").ap()
    cosq = nc.dram_tensor("cosq", [S, 128], BF16, kind="ExternalInput").ap()
    sinq = nc.dram_tensor("sinq", [S, 128], BF16, kind="ExternalInput").ap()
    cosk = nc.dram_tensor("cosk", [S, 64], BF16, kind="ExternalInput").ap()
    sink = nc.dram_tensor("sink", [S, 64], BF16, kind="ExternalInput").ap()
    out = nc.dram_tensor("out", [S, HID], BF16, kind="ExternalOutput").ap()
    dbg = None
    if debug:
        dbg = {}
        for t in range(4):
            dbg[f"qT{t}"] = nc.dram_tensor(f"dbg_qT{t}", [P, S], BF16, kind="ExternalOutput").ap()
            dbg[f"cT{t}"] = nc.dram_tensor(f"dbg_cT{t}", [P, S], BF16, kind="ExternalOutput").ap()
        for k in range(NKV):
            dbg[f"kT{k}"] = nc.dram_tensor(f"dbg_kT{k}", [P, S], BF16, kind="ExternalOutput").ap()
        dbg["va"] = nc.dram_tensor("dbg_va", [P, NKV * SB * 65], BF16, kind="ExternalOutput").ap()
    with tile.TileContext(nc) as tc:
        _emit_graph(nc, tc, xT, wqkv, wo, cosq, sinq, cosk, sink, out, dbg)
    nc.finalize()
    return nc


_DEINT = np.concatenate([np.arange(0, D, 2), np.arange(1, D, 2)])  # per-head perm


def _deint_cols(w, nheads):
    """permute last-dim columns: per head, evens then odds."""
    cols = np.concatenate([h * D + _DEINT for h in range(nheads)])
    return w[:, cols]


def kernel(x, wq, wk, wv, wo, freqs_cos, freqs_sin, mask):
    x = np.asarray(x, dtype=np.float32)
    wq = np.asarray(wq, dtype=np.float32)
    wk = np.asarray(wk, dtype=np.float32)
    wv = np.asarray(wv, dtype=np.float32)
    wo = np.asarray(wo, dtype=np.float32)
    fc = np.asarray(freqs_cos, dtype=np.float32)
    fs = np.asarray(freqs_sin, dtype=np.float32)

    if "nc" not in _CACHE:
        _CACHE["nc"] = _build()
    nc = _CACHE["nc"]

    inv = np.float32(1.0 / np.sqrt(np.float32(D)))
    cosq = np.ascontiguousarray(np.tile(fc * inv, (1, 4))).astype(BF)
    sinq = np.ascontiguousarray(np.tile(fs * inv, (1, 4))).astype(BF)
    cosk = np.ascontiguousarray(np.tile(fc, (1, 2))).astype(BF)
    sink = np.ascontiguousarray(np.tile(fs, (1, 2))).astype(BF)
    in_maps = []
    for core in range(8):
        b, t = core // 4, core % 4
        in_maps.append({
            "xT": np.ascontiguousarray(x[b].T).astype(BF),
            "wqkv": np.ascontiguousarray(np.concatenate(
                [_deint_cols(wk[:, t * KW:(t + 1) * KW], NKV),
                 wv[:, t * VW:(t + 1) * VW],
                 _deint_cols(wq[:, t * QW:(t + 1) * QW], NQ)], axis=1)).astype(BF),
            "wo": np.ascontiguousarray(wo[t * QW:(t + 1) * QW, :]).astype(BF),
            "cosq": cosq, "sinq": sinq, "cosk": cosk, "sink": sink,
        })
    trace = bool(_CACHE.get("trace"))
    try:
        res = run_bass_kernel_spmd(nc, in_maps, list(range(8)), trace=trace)
    except Exception:
        if not trace:
            raise
        res = run_bass_kernel_spmd(nc, in_maps, list(range(8)))
    _CACHE["last_result"] = res
    outs = [np.asarray(r["out"], dtype=np.float32) for r in res.results]
    full = np.stack([outs[0] + outs[1] + outs[2] + outs[3],
                     outs[4] + outs[5] + outs[6] + outs[7]], axis=0)
    return full


if __name__ == "__main__":
    import sys
    if "--build" in sys.argv:
        _build()
        print("build OK")


# revision 22
# speedup vs baseline: 1.0163x; 1.0044x over previous
"""Distributed Bass kernel for llama-style GQA attention on 8 trn2 NeuronCores.

Sharding: 2-way data-parallel over batch x 4-way tensor-parallel over heads.
Core c handles batch b=c//4 and head group t=c%4 (8 q-heads, 2 kv-heads).
wq/wk/wv split column-wise per head group; wo split row-wise; each core
produces a partial [S, HIDDEN] output (bf16), host sums the 4 partials.

Key structure (all matmuls bf16, psum f32):
  xT (pre-transposed on host) @ wqkv -> q,k,v  [seq partition-major].
  Phase A is DMA-paced chunk-major: as each xT/wqkv hidden-chunk arrives,
  the kv projections of the first two 4-seq-block super-units and the q
  projections of the first advance, so the PE tracks the DMA stream.
  RoPE in de-interleaved layout (host permutes wq/wk columns so even/odd
  rope halves are contiguous; cos/sin replicated per head on host, bf16).
  PE-transpose q,k to [d, seq]; q head pairs share one [128, S] tile
  (head A dims on partitions 0:64, head B on 64:128); kv heads duplicated
  on both halves so score matmuls ROW-PACK: two concurrent K=64 matmuls
  at array rows 0/64 (auto tile_position from base partitions).
  exp over a [128, 1024] pair tile in ONE activation per (pair, kb);
  causal = narrowed matmul/exp/ctx ranges + gpsimd affine_select on the
  diagonal 128x128 blocks (no memsets, no masked-garbage exp).
  ctx via vaug [v|ones] M=65, the pair shares the stationary operand;
  softmax denominators fall out in row 64; normalize = ACT shift-copy of
  the den row to partition 0 + K=1 ones-matmul broadcast + DVE fast
  reciprocal + one DVE mul (custom DVE ops and gpsimd partition ops
  misbehave at base partition != 0 on HW).
  out = ctxT.T @ wo in one full-K pass (no DRAM round-trip); wo and the
  second q-projection are interleaved one step per attention kb via
  generator fillers to keep the PE dense and HAM-warm; the wo tail
  alternates psum pools and evacuation engines to stay dense.
"""

import numpy as np
import ml_dtypes

import concourse.bass as bass
import concourse.mybir as mybir
import concourse.tile as tile
from concourse import bacc
from concourse.bass_utils import run_bass_kernel_spmd
from concourse.masks import make_identity

B, S, HID = 2, 2048, 2048
D = 64
NQ, NKV = 8, 2          # per-core heads
QW, KW, VW = NQ * D, NKV * D, NKV * D
QKVW = QW + KW + VW     # 768
P = 128
SB = S // P             # 16 seq blocks
KC = HID // P           # 16 contraction chunks
NSU = 4                 # seq super-units (4 sb each)
F32 = mybir.dt.float32
BF16 = mybir.dt.bfloat16
BF = ml_dtypes.bfloat16
AF = mybir.ActivationFunctionType
_CACHE = {}


def _emit_graph(nc, tc, xT, wqkv, wo, cosq, sinq, cosk, sink, out, dbg=None):
    with tc.tile_pool(name="const", bufs=1) as const, \
         tc.tile_pool(name="big", bufs=1) as big, \
         tc.tile_pool(name="wq_p", bufs=1) as wq_p, \
         tc.tile_pool(name="rt_p", bufs=2) as rt_p, \
         tc.tile_pool(name="exs", bufs=2) as exs_p, \
         tc.tile_pool(name="nrm", bufs=2) as nrm_p, \
         tc.tile_pool(name="osb", bufs=2) as osb_p:
        # persistent tiles
        qT_sb = [big.tile([P, S], BF16, tag=f"qT{t}", name=f"qT{t}") for t in range(4)]
        kT_sb = [big.tile([P, S], BF16, tag=f"kT{k}", name=f"kT{k}") for k in range(NKV)]
        vaug_sb = big.tile([P, NKV * SB * 65], BF16, tag="va")
        ctxT_sb = [big.tile([P, S], BF16, tag=f"cT{t}", name=f"cT{t}") for t in range(4)]

        ident = const.tile([P, P], BF16, tag="id")
        make_identity(nc, ident[:, :])
        nc.gpsimd.memset(vaug_sb[:, :], 1.0)
        ones64 = const.tile([1, D], BF16, tag="on")
        nc.gpsimd.memset(ones64[:, :], 1.0)

        xT_sb = wq_p.tile([P, KC * S], BF16, tag="xT")
        wqkv_sb = wq_p.tile([P, KC * QKVW], BF16, tag="wqkv")
        wo_sb = wq_p.tile([P, 4 * HID], BF16, tag="wo")
        cosq_sb = wq_p.tile([P, SB * 256], BF16, tag="cq")
        sinq_sb = wq_p.tile([P, SB * 256], BF16, tag="sq")
        cosk_sb = wq_p.tile([P, SB * 64], BF16, tag="ck")
        sink_sb = wq_p.tile([P, SB * 64], BF16, tag="sk")
        # critical-path DMAs first: per hidden-chunk, xT then the kv+qg0
        # slice of wqkv; everything else queued after.
        for kc in range(KC):
            nc.sync.dma_start(out=xT_sb[:, kc * S:(kc + 1) * S], in_=xT[kc * P:(kc + 1) * P, :])
            nc.sync.dma_start(out=wqkv_sb[:, kc * QKVW:kc * QKVW + 512],
                              in_=wqkv[kc * P:(kc + 1) * P, 0:512])
        for sb in range(SB):
            nc.sync.dma_start(out=cosk_sb[:, sb * 64:(sb + 1) * 64], in_=cosk[sb * P:(sb + 1) * P, :])
            nc.sync.dma_start(out=sink_sb[:, sb * 64:(sb + 1) * 64], in_=sink[sb * P:(sb + 1) * P, :])
            nc.sync.dma_start(out=cosq_sb[:, sb * 256:(sb + 1) * 256], in_=cosq[sb * P:(sb + 1) * P, :])
            nc.sync.dma_start(out=sinq_sb[:, sb * 256:(sb + 1) * 256], in_=sinq[sb * P:(sb + 1) * P, :])
        for kc in range(KC):
            nc.sync.dma_start(out=wqkv_sb[:, kc * QKVW + 512:(kc + 1) * QKVW],
                              in_=wqkv[kc * P:(kc + 1) * P, 512:QKVW])
        for c in range(4):
            nc.sync.dma_start(out=wo_sb[:, c * HID:(c + 1) * HID], in_=wo[c * P:(c + 1) * P, :])

        # ------------- rope (de-interleaved layout, 4-sb batched) -------------
        def rope_q(ps, h2, dst):
            """ps: psum [P, 1024] (2 sb x 8 heads x [e|o]32); dst same, bf16."""
            n = 16
            t1 = rt_p.tile([P, 512], F32, tag="t1", name="t1")
            t2 = rt_p.tile([P, 512], F32, tag="t2", name="t2")
            ev = ps.rearrange("p (sh x) -> p sh x", sh=n)[:, :, 0:32]
            od = ps.rearrange("p (sh x) -> p sh x", sh=n)[:, :, 32:64]
            de = dst.rearrange("p (sh x) -> p sh x", sh=n)[:, :, 0:32]
            do = dst.rearrange("p (sh x) -> p sh x", sh=n)[:, :, 32:64]
            c = cosq_sb[:, h2 * 512:(h2 + 1) * 512].rearrange("p (sh i) -> p sh i", sh=n)
            sn = sinq_sb[:, h2 * 512:(h2 + 1) * 512].rearrange("p (sh i) -> p sh i", sh=n)
            t1r = t1[:, :].rearrange("p (sh i) -> p sh i", sh=n)
            t2r = t2[:, :].rearrange("p (sh i) -> p sh i", sh=n)
            nc.vector.tensor_mul(t1r, ev, c)
            nc.vector.tensor_mul(t2r, od, sn)
            nc.vector.tensor_sub(de, t1r, t2r)
            nc.vector.tensor_mul(t1r, ev, sn)
            nc.vector.tensor_mul(t2r, od, c)
            nc.vector.tensor_add(do, t1r, t2r)

        def rope_k(ps, su, dst):
            """ps: psum [P, 1024] (4 sb x [k: 2 kv x [e|o]32 | v: 128]);
            dst: krot [P, 512] (4 sb x 2 kv x 64), bf16. 4D views."""
            t1f = rt_p.tile([P, 512], F32, tag="t1", name="t1k")
            t2f = rt_p.tile([P, 512], F32, tag="t2", name="t2k")
            t1, t2 = t1f[:, 0:256], t2f[:, 0:256]
            kview = ps.rearrange("p (s c) -> p s c", s=4)
            ev = kview[:, :, 0:128].rearrange("p s (h x) -> p s h x", h=2)[:, :, :, 0:32]
            od = kview[:, :, 0:128].rearrange("p s (h x) -> p s h x", h=2)[:, :, :, 32:64]
            dv = dst.rearrange("p (s c) -> p s c", s=4)
            de = dv.rearrange("p s (h x) -> p s h x", h=2)[:, :, :, 0:32]
            do = dv.rearrange("p s (h x) -> p s h x", h=2)[:, :, :, 32:64]
            c = cosk_sb[:, su * 256:(su + 1) * 256].rearrange("p (s h i) -> p s h i", s=4, h=2)
            sn = sink_sb[:, su * 256:(su + 1) * 256].rearrange("p (s h i) -> p s h i", s=4, h=2)
            t1r = t1.rearrange("p (s h i) -> p s h i", s=4, h=2)
            t2r = t2.rearrange("p (s h i) -> p s h i", s=4, h=2)
            nc.vector.tensor_mul(t1r, ev, c)
            nc.vector.tensor_mul(t2r, od, sn)
            nc.vector.tensor_sub(de, t1r, t2r)
            nc.vector.tensor_mul(t1r, ev, sn)
            nc.vector.tensor_mul(t2r, od, c)
            nc.vector.tensor_add(do, t1r, t2r)

        # ------------- projection pieces --------------------------------------
        # NOTE: at most ONE open psum accumulation group per bank. A [P, 1024]
        # su tile spans 2 banks (regions i=0,1 in bank 0; i=2,3 in bank 1), so
        # chunk-major passes interleave regions {0, 2} then {1, 3}.
        def kv_mm_one(ps, su, i, kc):
            sb = su * 4 + i
            nc.tensor.matmul(ps[:, i * 256:(i + 1) * 256],
                             xT_sb[:, kc * S + sb * P: kc * S + (sb + 1) * P],
                             wqkv_sb[:, kc * QKVW: kc * QKVW + 256],
                             start=(kc == 0), stop=(kc == KC - 1))

        def q_mm_one(ps, h2, i, kc):
            """h2: 2-sb half-super-unit index (sb = 2*h2 + i); one N=512
            matmul projects ALL 8 q heads of one seq block."""
            sb = h2 * 2 + i
            nc.tensor.matmul(ps[:, i * 512:(i + 1) * 512],
                             xT_sb[:, kc * S + sb * P: kc * S + (sb + 1) * P],
                             wqkv_sb[:, kc * QKVW + 256: kc * QKVW + 768],
                             start=(kc == 0), stop=(kc == KC - 1))

        def kv_mms(ps, su):
            for i in range(4):
                for kc in range(KC):
                    kv_mm_one(ps, su, i, kc)

        def q_mms(ps, h2):
            for i in range(2):
                for kc in range(KC):
                    q_mm_one(ps, h2, i, kc)

        def kv_side(ps, su):
            """rope k; evac v (DVE)."""
            krot = rt_p.tile([P, 512], BF16, tag="krot", name="krot")
            rope_k(ps, su, krot[:, :])
            vsrc = ps.rearrange("p (s c) -> p s c", s=4)
            for kv in range(NKV):
                vdst = vaug_sb[:, kv * SB * 65 + su * 4 * 65:
                               kv * SB * 65 + (su * 4 + 4) * 65].rearrange(
                    "p (s d) -> p s d", s=4)
                nc.vector.tensor_copy(vdst[:, :, 0:D],
                                      vsrc[:, :, 128 + kv * D:128 + (kv + 1) * D])
            return krot

        def kv_transp(krot, su, ptpool, pttag):
            for i in range(4):
                sb = su * 4 + i
                pt = ptpool.tile([P, P], BF16, tag=pttag, name="pt")
                nc.tensor.transpose(pt[:], krot[:, i * 128:(i + 1) * 128], ident[:, :])
                nc.vector.tensor_copy(kT_sb[0][0:D, sb * P:(sb + 1) * P], pt[0:D, :])
                nc.vector.tensor_copy(kT_sb[1][D:P, sb * P:(sb + 1) * P], pt[D:P, :])
                nc.sync.dma_start(out=kT_sb[0][D:P, sb * P:(sb + 1) * P],
                                  in_=kT_sb[0][0:D, sb * P:(sb + 1) * P])
                nc.sync.dma_start(out=kT_sb[1][0:D, sb * P:(sb + 1) * P],
                                  in_=kT_sb[1][D:P, sb * P:(sb + 1) * P])

        def q_side(ps, h2):
            qrot = rt_p.tile([P, 1024], BF16, tag="qrot", name="qrot")
            rope_q(ps, h2, qrot[:, :])
            return qrot

        def q_transp(qrot, h2, ptpool, pttag):
            for i in range(2):
                sb = h2 * 2 + i
                for c in range(4):
                    pt = ptpool.tile([P, P], BF16, tag=pttag, name="pt")
                    nc.tensor.transpose(pt[:], qrot[:, i * 512 + c * P: i * 512 + (c + 1) * P],
                                        ident[:, :])
                    nc.vector.tensor_copy(qT_sb[c][:, sb * P:(sb + 1) * P], pt[:])

        def q_h2_filler(pool, h2, chunk=8):
            """generator: deferred q projection (2 seq blocks) as filler."""
            ps = pool.tile([P, 1024], F32, tag="f", name="qps")
            nmm = 0
            for i in range(2):
                for kc in range(KC):
                    q_mm_one(ps, h2, i, kc)
                    nmm += 1
                    if nmm % chunk == 0:
                        yield
            qrot = q_side(ps, h2)
            # let the rope (DVE) drain while attention proceeds before
            # emitting PE transposes that wait on it
            for _ in range(4):
                yield
            for i in range(2):
                sb = h2 * 2 + i
                for c in range(4):
                    pt = pool.tile([P, P], BF16, tag="f", name="pt")
                    nc.tensor.transpose(pt[:], qrot[:, i * 512 + c * P: i * 512 + (c + 1) * P],
                                        ident[:, :])
                    nc.vector.tensor_copy(qT_sb[c][:, sb * P:(sb + 1) * P], pt[:])
                yield

        def wo_unit(pool, tag, sb, n, evac):
            po = pool.tile([P, 1024], F32, tag=tag, name="po")
            for c in range(4):
                nc.tensor.matmul(po[:, 0:512], ctxT_sb[c][:, sb * P:(sb + 1) * P],
                                 wo_sb[:, c * HID + n * 512: c * HID + (n + 1) * 512],
                                 start=(c == 0), stop=(c == 3))
                if c == 1:
                    yield
            ob = osb_p.tile([P, 512], BF16, tag="ob", name="ob")
            if evac == "s":
                nc.scalar.activation(ob[:, :], po[:, 0:512], AF.Copy)
            else:
                nc.vector.tensor_copy(ob[:], po[:, 0:512])
            nc.sync.dma_start(out=out[sb * P:(sb + 1) * P, n * 512:(n + 1) * 512], in_=ob[:])

        def next_filler(filler):
            while filler:
                try:
                    next(filler[0])
                    return True
                except StopIteration:
                    filler.pop(0)
            return False

        # ------------- phase A ------------------------------------------------
        with tc.tile_pool(name="psA", bufs=1, space="PSUM") as psA, \
             tc.tile_pool(name="psAt", bufs=2, space="PSUM") as psAt:
            kvp0 = psA.tile([P, 1024], F32, tag="kv0", name="kvps0")
            kvp1 = psA.tile([P, 1024], F32, tag="kv1", name="kvps1")
            qp0 = psA.tile([P, 1024], F32, tag="q", name="qps0")
            # pass 1: chunk-major over bank-disjoint regions (one open psum
            # group per bank) — the PE tracks the xT/wqkv DMA arrival order
            for kc in range(KC):
                for ps_, su in ((kvp0, 0), (kvp1, 1)):
                    kv_mm_one(ps_, su, 0, kc)
                    kv_mm_one(ps_, su, 2, kc)
                q_mm_one(qp0, 0, 0, kc)
                q_mm_one(qp0, 0, 1, kc)
            qr0 = q_side(qp0, 0)              # DVE-only
            # pass 2: kv regions {1, 3} chunk-major, then q h2=1 dense
            qp1 = psA.tile([P, 1024], F32, tag="q", name="qps1")
            for kc in range(KC):
                for ps_, su in ((kvp0, 0), (kvp1, 1)):
                    kv_mm_one(ps_, su, 1, kc)
                    kv_mm_one(ps_, su, 3, kc)
            q_mms(qp1, 1)
            kr0 = kv_side(kvp0, 0)
            qr1 = q_side(qp1, 1)
            kvp2 = psA.tile([P, 1024], F32, tag="kv0", name="kvps2")
            kv_mms(kvp2, 2)                   # PE dense while ropes drain
            kr1 = kv_side(kvp1, 1)
            qp2 = psA.tile([P, 1024], F32, tag="q", name="qps2")
            q_mms(qp2, 2)
            kv_transp(kr0, 0, psAt, "pt")
            kv_transp(kr1, 1, psAt, "pt")
            q_transp(qr0, 0, psAt, "pt")
            kvp3 = psA.tile([P, 1024], F32, tag="kv1", name="kvps3")
            kv_mms(kvp3, 3)
            kr2 = kv_side(kvp2, 2)
            qr2 = q_side(qp2, 2)
            qp3 = psA.tile([P, 1024], F32, tag="q", name="qps3")
            q_mms(qp3, 3)
            kv_transp(kr2, 2, psAt, "pt")
            q_transp(qr1, 1, psAt, "pt")
            kr3 = kv_side(kvp3, 3)
            qr3 = q_side(qp3, 3)
            qp4 = psA.tile([P, 1024], F32, tag="q", name="qps4")
            q_mms(qp4, 4)
            kv_transp(kr3, 3, psAt, "pt")
            q_transp(qr2, 2, psAt, "pt")
            qr4 = q_side(qp4, 4)
            qp5 = psA.tile([P, 1024], F32, tag="q", name="qps5")
            q_mms(qp5, 5)
            q_transp(qr3, 3, psAt, "pt")
            qr5 = q_side(qp5, 5)
            q_transp(qr4, 4, psAt, "pt")
            q_transp(qr5, 5, psAt, "pt")

        # ------------- attention + qg1 + wo, interleaved ----------------------
        with tc.tile_pool(name="pss", bufs=2, space="PSUM") as pss_p, \
             tc.tile_pool(name="psc", bufs=1, space="PSUM") as psc_p, \
             tc.tile_pool(name="psF", bufs=1, space="PSUM") as psF:

            def attn_unit(t, qb, filler):
                """pair t (heads 2t, 2t+1), q block qb (512 wide)."""
                kv = t // 2
                nkb = min(4 * qb + 4, SB)
                ctx = psc_p.tile([P, 1024], F32, tag="ctx", name="ctx")
                for kb in range(nkb):
                    w0 = max(kb - 4 * qb, 0) * P
                    sps = pss_p.tile([P, 1024], F32, tag="sT", name="sT")
                    nc.tensor.matmul(sps[:, w0:512],
                                     kT_sb[kv][0:D, kb * P:(kb + 1) * P],
                                     qT_sb[t][0:D, qb * 512 + w0:(qb + 1) * 512],
                                     start=True, stop=True)
                    nc.tensor.matmul(sps[:, 512 + w0:1024],
                                     kT_sb[kv][D:P, kb * P:(kb + 1) * P],
                                     qT_sb[t][D:P, qb * 512 + w0:(qb + 1) * 512],
                                     start=True, stop=True)
                    ex = exs_p.tile([P, 1024], BF16, tag="ex", name="ex")
                    nc.scalar.activation(
                        ex.rearrange("p (h q) -> p h q", h=2)[:, :, w0:512],
                        sps.rearrange("p (h q) -> p h q", h=2)[:, :, w0:512],
                        AF.Exp)
                    if kb >= 4 * qb:     # diagonal 128x128 sub-block
                        jc = (kb - 4 * qb) * P
                        for h in range(2):
                            nc.gpsimd.affine_select(
                                out=ex[:, h * 512 + jc:h * 512 + jc + P],
                                in_=ex[:, h * 512 + jc:h * 512 + jc + P],
                                compare_op=mybir.AluOpType.is_ge,
                                fill=0.0, base=0, pattern=[[1, P]], channel_multiplier=-1)
                    next_filler(filler)
                    va = vaug_sb[:, kv * SB * 65 + kb * 65: kv * SB * 65 + kb * 65 + 65]
                    nc.tensor.matmul(ctx[0:65, w0:512], va, ex[:, w0:512],
                                     start=(kb == 0), stop=(kb == nkb - 1))
                    nc.tensor.matmul(ctx[0:65, 512 + w0:1024], va, ex[:, 512 + w0:1024],
                                     start=(kb == 0), stop=(kb == nkb - 1))
                # normalize: den = ctx row 64 -> ACT shift to partition 0 ->
                # ones-matmul broadcast -> DVE recip + mul (base 0 only).
                den = nrm_p.tile([1, 1024], BF16, tag="den", name="den")
                nc.scalar.activation(den[0:1, :], ctx[64:65, :], AF.Copy)
                # bz lives in the scores ring: its prior occupant's readers
                # (exp) are always already emitted, unlike the psF ring whose
                # occupant may be a filler projection awaiting its rope.
                bz = pss_p.tile([P, 1024], F32, tag="sT", name="bz")
                nc.tensor.matmul(bz[0:D, 0:512], ones64[0:1, :], den[0:1, 0:512],
                                 start=True, stop=True)
                nc.tensor.matmul(bz[0:D, 512:1024], ones64[0:1, :], den[0:1, 512:1024],
                                 start=True, stop=True)
                bcs = nrm_p.tile([D, 1024], F32, tag="bcs", name="bcs")
                nc.vector.reciprocal_approx_fast(out=bcs[0:D, :], in_=bz[0:D, :])
                ntmp = nrm_p.tile([D, 1024], BF16, tag="ntmp", name="ntmp")
                nc.vector.tensor_mul(ntmp[:, :], ctx[0:D, :], bcs[0:D, :])
                nc.sync.dma_start(out=ctxT_sb[t][0:D, qb * 512:(qb + 1) * 512],
                                  in_=ntmp[:, 0:512])
                nc.sync.dma_start(out=ctxT_sb[t][D:P, qb * 512:(qb + 1) * 512],
                                  in_=ntmp[:, 512:1024])

            # q h2 units 6, 7 (seq blocks 12-15, only needed by qb=3) are
            # deferred into the attention stream as its first fillers
            filler = [q_h2_filler(psF, 6), q_h2_filler(psF, 7)]
            order = [(t, qb) for qb in range(4) for t in range(4)]
            for t, qb in order:
                attn_unit(t, qb, filler)
                if t == 3 and qb < 3:
                    for n in range(4):
                        for sb in range(4 * qb, 4 * qb + 4):
                            filler.append(wo_unit(psF, "f", sb, n, "v"))
            while next_filler(filler):
                pass
            # dense tail: last qb's wo alternates between two psum rings
            tail = []
            for i, (n, sb) in enumerate([(n, sb) for n in range(4) for sb in range(12, 16)]):
                pool, tag = (psF, "f") if i % 2 == 0 else (pss_p, "sT")
                tail.append(wo_unit(pool, tag, sb, n, "v"))
            while next_filler(tail):
                pass

        if dbg:
            for t in range(4):
                nc.sync.dma_start(out=dbg[f"qT{t}"], in_=qT_sb[t][:, :])
                nc.sync.dma_start(out=dbg[f"cT{t}"], in_=ctxT_sb[t][:, :])
            for k in range(NKV):
                nc.sync.dma_start(out=dbg[f"kT{k}"], in_=kT_sb[k][:, :])
            nc.sync.dma_start(out=dbg["va"], in_=vaug_sb[:, :])


def _build(debug=False):
    nc = bacc.Bacc("TRN2", target_bir_lowering=False, debug=False, num_devices=8)
    xT = nc.dram_tensor("xT", [HID, S], BF16, kind="ExternalInput").ap()
    wqkv = nc.dram_tensor("wqkv", [HID, QKVW], BF16, kind="ExternalInput").ap()
    wo = nc.dram_tensor("wo", [QW, HID], BF16, kind="ExternalInput").ap()
    cosq = nc.dram_tensor("cosq", [S, 256], BF16, kind="ExternalInput").ap()
    sinq = nc.dram_tensor("sinq", [S, 256], BF16, kind="ExternalInput").ap()
    cosk = nc.dram_tensor("cosk", [S, 64], BF16, kind="ExternalInput").ap()
    sink = nc.dram_tensor("sink", [S, 64], BF16, kind="ExternalInput").ap()
    out = nc.dram_tensor("out", [S, HID], BF16, kind="ExternalOutput").ap()
    dbg = None
    if debug:
        dbg = {}
        for t in range(4):
            dbg[f"qT{t}"] = nc.dram_tensor(f"dbg_qT{t}", [P, S], BF16, kind="ExternalOutput").ap()
            dbg[f"cT{t}"] = nc.dram_tensor(f"dbg_cT{t}", [P, S], BF16, kind="ExternalOutput").ap()
        for k in range(NKV):
            dbg[f"kT{k}"] = nc.dram_tensor(f"dbg_kT{k}", [P, S], BF16, kind="ExternalOutput").ap()
        dbg["va"] = nc.dram_tensor("dbg_va", [P, NKV * SB * 65], BF16, kind="ExternalOutput").ap()
    with tile.TileContext(nc) as tc:
        _emit_graph(nc, tc, xT, wqkv, wo, cosq, sinq, cosk, sink, out, dbg)
    nc.finalize()
    return nc


_DEINT = np.concatenate([np.arange(0, D, 2), np.arange(1, D, 2)])  # per-head perm


def _deint_cols(w, nheads):
    """permute last-dim columns: per head, evens then odds."""
    cols = np.concatenate([h * D + _DEINT for h in range(nheads)])
    return w[:, cols]


def kernel(x, wq, wk, wv, wo, freqs_cos, freqs_sin, mask):
    x = np.asarray(x, dtype=np.float32)
    wq = np.asarray(wq, dtype=np.float32)
    wk = np.asarray(wk, dtype=np.float32)
    wv = np.asarray(wv, dtype=np.float32)
    wo = np.asarray(wo, dtype=np.float32)
    fc = np.asarray(freqs_cos, dtype=np.float32)
    fs = np.asarray(freqs_sin, dtype=np.float32)

    if "nc" not in _CACHE:
        _CACHE["nc"] = _build()
    nc = _CACHE["nc"]

    inv = np.float32(1.0 / np.sqrt(np.float32(D)))
    cosq = np.ascontiguousarray(np.tile(fc * inv, (1, 8))).astype(BF)
    sinq = np.ascontiguousarray(np.tile(fs * inv, (1, 8))).astype(BF)
    cosk = np.ascontiguousarray(np.tile(fc, (1, 2))).astype(BF)
    sink = np.ascontiguousarray(np.tile(fs, (1, 2))).astype(BF)
    in_maps = []
    for core in range(8):
        b, t = core // 4, core % 4
        in_maps.append({
            "xT": np.ascontiguousarray(x[b].T).astype(BF),
            "wqkv": np.ascontiguousarray(np.concatenate(
                [_deint_cols(wk[:, t * KW:(t + 1) * KW], NKV),
                 wv[:, t * VW:(t + 1) * VW],
                 _deint_cols(wq[:, t * QW:(t + 1) * QW], NQ)], axis=1)).astype(BF),
            "wo": np.ascontiguousarray(wo[t * QW:(t + 1) * QW, :]).astype(BF),
            "cosq": cosq, "sinq": sinq, "cosk": cosk, "sink": sink,
        })
    trace = bool(_CACHE.get("trace"))
    try:
        res = run_bass_kernel_spmd(nc, in_maps, list(range(8)), trace=trace)
    except Exception:
        if not trace:
            raise
        res = run_bass_kernel_spmd(nc, in_maps, list(range(8)))
    _CACHE["last_result"] = res
    outs = [np.asarray(r["out"], dtype=np.float32) for r in res.results]
    full = np.stack([outs[0] + outs[1] + outs[2] + outs[3],
                     outs[4] + outs[5] + outs[6] + outs[7]], axis=0)
    return full


if __name__ == "__main__":
    import sys
    if "--build" in sys.argv:
        _build()
        print("build OK")
